# revision 8
# baseline (speedup 1.0000x reference)
"""Trainium2 Bass kernel for nn_AsyncConv (geodesic directional conv + max-pool).

Reference computation:
    g = take(y, exp_map, axis=1)                  # (B, NV, NR, ND, C)
    g = wrap-pad dirs to 2*ND-1
    out = conv_valid(g, kernel) + bias; relu      # (B*NV, ND, NF)
    out = max over ND                             # (B, NV, NF)

Reformulated as one dense matmul per vertex tile:
    out[v, (d,f)] = sum_{r,j,c} g[v,r,j,c] * kernel[r,(j-d)%ND,c,f]
i.e. OUT = G @ W with G (rows=(b,v), cols=(r,j,c)) and
W[(r,j,c),(d,f)] = kernel[r,(j-d)%ND,c,f], then relu(+bias) and max over d.

Sharding: vertex-parallel across 8 cores; W/bias replicated.
The gather G is materialized on the host (numpy fancy indexing) into
pre-transposed bf16 lhsT tiles; the device does the 503 GFLOP matmul,
direction max-fold, bias and relu.
"""

import sys
import os

sys.path.insert(0, "/opt/trn_rl_repo")

import numpy as np

import concourse.bass as bass
import concourse.mybir as mybir
from concourse.tile import TileContext
from concourse.vector_clock import ScopedClock
from concourse.bass_utils import run_bass_kernel_spmd

import ml_dtypes

BF16 = ml_dtypes.bfloat16

# problem constants (hardcoded per harness contract)
B, NV, C = 2, 20000, 64
NRINGS, NDIRS, NF = 3, 16, 128
NCORES = 8
NV_LOCAL = NV // NCORES            # 2500
ROWS_LOCAL = B * NV_LOCAL          # 5000
P = 128
NTILES = (ROWS_LOCAL + P - 1) // P  # 40 (last tile padded with 120 dummy rows)
KDIM = NRINGS * NDIRS * C          # 3072
KT = KDIM // P                     # 24 k-tiles
NDIM = NDIRS * NF                  # 2048
NCHUNK = 512                       # psum bank free size (fp32)
NNC = NDIM // NCHUNK               # 4 n-chunks


_WS_COUNTER = [0]


def _split_sync_waits(nc, max_waits=1):
    """This walrus build rejects instructions with more than ~1-2 sync waits
    ("Too many sync wait commands"). Hoist excess waits onto NOP
    instructions inserted immediately before the offending instruction on
    the same engine — waits execute in order, so semantics are unchanged."""
    for f in nc.m.functions:
        for bb in f.blocks:
            new_insts = []
            changed = False
            for inst in bb.instructions:
                si = getattr(inst, "sync_info", None)
                ow = list(si.on_wait) if si is not None else []
                if len(ow) > max_waits:
                    SyncInfo = type(si)
                    # keep the LAST max_waits on the instruction; earlier
                    # waits go onto preceding NOPs in order
                    excess, keep = ow[:-max_waits], ow[-max_waits:]
                    for i in range(0, len(excess), max_waits):
                        _WS_COUNTER[0] += 1
                        nop = mybir.InstNoOp(
                            name=f"I-wsplit-{_WS_COUNTER[0]}",
                            engine=inst.engine,
                            sync_info=SyncInfo(
                                on_wait=excess[i : i + max_waits], on_update=[]
                            ),
                            bass_nofuse=True,
                        )
                        new_insts.append(nop)
                    si.on_wait = keep
                    inst.sync_info = si
                    changed = True
                new_insts.append(inst)
            if changed:
                bb.instructions = new_insts


def build_nc():
    """Build the per-core Bass program (same SPMD graph on all 8 cores)."""
    nc = bass.Bass()
    f32 = mybir.dt.float32
    bf16 = mybir.dt.bfloat16

    gpatch = nc.declare_dram_parameter("gpatch", [NTILES, P, KDIM], bf16, isOutput=False)
    wmat = nc.declare_dram_parameter("wmat", [P, KT * NDIM], bf16, isOutput=False)
    bias_b = nc.declare_dram_parameter("bias_b", [P, NF], f32, isOutput=False)
    outp = nc.declare_dram_parameter("out", [NTILES, P, NF], f32, isOutput=True)

    with TileContext(nc) as tc:
        with (
            tc.tile_pool(name="wpool", bufs=1) as wpool,
            tc.tile_pool(name="gpool", bufs=3) as gpool,
            tc.tile_pool(name="apool", bufs=3) as apool,
            tc.tile_pool(name="psum", bufs=8, space="PSUM") as pspool,
        ):
            wsb = wpool.tile([P, KT * NDIM], bf16)
            nc.sync.dma_start(out=wsb[:], in_=wmat[:])
            bias_t = wpool.tile([P, NF], f32)
            nc.sync.dma_start(out=bias_t[:], in_=bias_b[:])

            for t in range(NTILES):
                gp = gpool.tile([P, KDIM], bf16, tag="gp")
                nc.sync.dma_start(out=gp[:], in_=gpatch[t])

                psums = []
                for n in range(NNC):
                    ps = pspool.tile([P, NCHUNK], f32, tag="ps")
                    for k in range(KT):
                        nc.tensor.matmul(
                            ps[:],
                            lhsT=gp[:, k * P : (k + 1) * P],
                            rhs=wsb[:, k * NDIM + n * NCHUNK : k * NDIM + n * NCHUNK + NCHUNK],
                            start=(k == 0),
                            stop=(k == KT - 1),
                        )
                    psums.append(ps)

                acc = apool.tile([P, NF], f32, tag="acc")
                # max over the 16 direction chunks (4 per psum bank)
                chunks = [
                    psums[n][:, j * NF : (j + 1) * NF]
                    for n in range(NNC)
                    for j in range(NCHUNK // NF)
                ]
                nc.vector.tensor_copy(out=acc[:], in_=chunks[0])
                for ch in chunks[1:]:
                    nc.vector.tensor_tensor(
                        out=acc[:], in0=acc[:], in1=ch, op=mybir.AluOpType.max
                    )
                # + bias, relu
                nc.vector.tensor_tensor(
                    out=acc[:], in0=acc[:], in1=bias_t[:], op=mybir.AluOpType.add
                )
                nc.vector.tensor_scalar_max(out=acc[:], in0=acc[:], scalar1=0.0)
                nc.sync.dma_start(out=outp[t], in_=acc[:])

    _split_sync_waits(nc)
    return nc


def host_prep(y, exp_map, kernel, bias):
    """Build per-core input maps: pre-gathered bf16 lhsT tiles + expanded W."""
    y = np.asarray(y, dtype=np.float32)
    exp_map = np.asarray(exp_map)
    kernel = np.asarray(kernel, dtype=np.float32)
    bias = np.asarray(bias, dtype=np.float32)

    # ---- expanded weight W[(r,j,c),(d,f)] = kernel[r,(j-d)%ND,c,f] ----
    # j_idx (ND, ND): [j, d] -> (j-d) % ND
    j_idx = (np.arange(NDIRS)[:, None] - np.arange(NDIRS)[None, :]) % NDIRS
    # kernel (NR, ND, C, NF) -> W (NR, ND_j, C, ND_d, NF)
    W = kernel[:, j_idx, :, :]            # (NR, ND_j, ND_d, C, NF)
    W = W.transpose(0, 1, 3, 2, 4)        # (NR, ND_j, C, ND_d, NF)
    W = W.reshape(KDIM, NDIM)             # ((r,j,c), (d,f))
    # device layout: (128 part, KT*NDIM) with tile k at cols [k*NDIM:(k+1)*NDIM]
    Wd = W.reshape(KT, P, NDIM).transpose(1, 0, 2).reshape(P, KT * NDIM)
    Wd = np.ascontiguousarray(Wd, dtype=BF16)

    bias_b = np.ascontiguousarray(np.broadcast_to(bias, (P, NF)), dtype=np.float32)

    # ---- per-core gathered patch tiles ----
    y_flat = y.reshape(B * NV, C)  # row (b,v) = b*NV + v
    in_maps = []
    for c in range(NCORES):
        v0 = c * NV_LOCAL
        # local row order: b-major then v  -> row r = b*NV_LOCAL + vl
        vl = np.arange(v0, v0 + NV_LOCAL)
        em = exp_map[vl].reshape(NV_LOCAL, NRINGS * NDIRS)   # (2500, 48)
        rows = np.concatenate(
            [em + b * NV for b in range(B)], axis=0
        )  # (5000, 48) indices into y_flat
        pad = NTILES * P - rows.shape[0]
        if pad:
            rows = np.concatenate([rows, np.zeros((pad, 48), dtype=rows.dtype)], axis=0)
        G = y_flat[rows]                     # (5120, 48, 64) f32
        G = G.astype(BF16)
        # DRAM layout: gpatch[t, p, k*128+v] = G[t*128+v, 2k + p//64, p%64]
        # i.e. partition p = (rj parity, channel), free = (ktile, vertex-in-tile)
        G = G.reshape(NTILES, P, KT, 2, C)           # (t, v, k, par, c)
        G = G.transpose(0, 3, 4, 2, 1)               # (t, par, c, k, v)
        G = np.ascontiguousarray(G).reshape(NTILES, P, KDIM)
        in_maps.append({"gpatch": G, "wmat": Wd, "bias_b": bias_b})
    return in_maps


def unshard(results):
    out = np.empty((B, NV, NF), dtype=np.float32)
    for c in range(NCORES):
        r = results[c]["out"].reshape(NTILES * P, NF)[:ROWS_LOCAL]
        for b in range(B):
            out[b, c * NV_LOCAL : (c + 1) * NV_LOCAL] = r[
                b * NV_LOCAL : (b + 1) * NV_LOCAL
            ]
    return out


def _install_profile_shim():
    """The agent image lacks ``antenv.axon_hooks``; recreate the tiny hook
    registry + the ctypes NTFF hook from trn_boot so trace=True works.
    Also neuter upload_artifacts (zero-egress container)."""
    import types, ctypes, contextlib
    import antenv
    from concourse import bass_utils as bu

    bu.upload_artifacts = lambda tmpdir: tmpdir  # no egress

    if "antenv.axon_hooks" in sys.modules:
        return
    mod = types.ModuleType("antenv.axon_hooks")
    _state = {"hook": None}
    mod.set_axon_ntff_profile_hook = lambda h: _state.__setitem__("hook", h)
    mod.get_axon_ntff_profile_hook = lambda: _state["hook"]
    sys.modules["antenv.axon_hooks"] = mod
    antenv.axon_hooks = mod

    so_path = "/opt/axon/libaxon_pjrt.so"
    lib = ctypes.CDLL(so_path)
    if not hasattr(lib, "axon_start_nrt_profile"):
        return
    lib.axon_start_nrt_profile.argtypes = [
        ctypes.POINTER(ctypes.c_int64),
        ctypes.c_size_t,
    ]
    lib.axon_start_nrt_profile.restype = ctypes.c_int64
    lib.axon_stop_nrt_profile.argtypes = [ctypes.c_char_p]
    lib.axon_stop_nrt_profile.restype = ctypes.c_int64

    @contextlib.contextmanager
    def _hook(output_dir, device_ids):
        import jax

        jax.devices()
        if device_ids:
            ids = (ctypes.c_int64 * len(device_ids))(*device_ids)
            rc = lib.axon_start_nrt_profile(ids, len(device_ids))
        else:
            rc = lib.axon_start_nrt_profile(None, 0)
        if rc != 0:
            raise RuntimeError(f"axon_start_nrt_profile rc={rc}")
        try:
            yield
        finally:
            n = lib.axon_stop_nrt_profile(str(output_dir).encode())
            print(f"profile: {n} file(s) written to {output_dir}")

    mod.set_axon_ntff_profile_hook(_hook)


def run(y, exp_map, kernel, bias, trace=False):
    if trace:
        _install_profile_shim()
    nc = build_nc()
    in_maps = host_prep(y, exp_map, kernel, bias)
    res = run_bass_kernel_spmd(
        nc, in_maps, core_ids=list(range(NCORES)), trace=trace
    )
    return unshard(res.results), res


def kernel(y, exp_map, kernel, bias):  # noqa: A002 - name fixed by contract
    out, _ = run(y, exp_map, kernel, bias, trace=False)
    return out
